# revision 1
# baseline (speedup 1.0000x reference)
"""Trainium2 Bass kernel for nn_C3S_RegularLoss.

reference:
    xr = x.reshape(B, P, D); xn = xr / ||xr||_2(axis=-1)
    s = mean_b(xn)                     # (P, D)
    corr = s @ s.T                     # (P, P)
    loss = (sum(corr) - 3*trace(corr) + 2P) / 2 * gamma

Reformulated without the corr matrix:
    sum(corr)   = || sum_p s_p ||^2
    trace(corr) = sum_p || s_p ||^2
so with S = sum_b xn (sum, not mean):
    loss = ((||sum_p S_p||^2 - 3*sum(S^2)) / B^2 + 2P) / 2 * gamma

Sharding: data-parallel over the batch dim, 8 cores x 1024 rows.
Each core computes S_partial = sum_b r_b * x_b per part via PE matmuls
(r = 1/||x_part|| as the stationary operand), AllReduce of the (4,2048)
sums, then a tiny replicated tail computes the scalar loss.
"""

import os
import sys

sys.path.insert(0, "/opt/trn_rl_repo")
os.environ.setdefault("MYCRO_LOCAL_CACHE", "1")

import numpy as np

B, F = 8192, 8192
NPARTS = 4
D = F // NPARTS                 # 2048
NCORES = 8
B_CORE = B // NCORES            # 1024
TILE_P = 128
NTILES = B_CORE // TILE_P       # 8
MM_N = 512                      # moving free dim per matmul
NCHUNK = D // MM_N              # 4

_cache = {}


def _build(ncores=NCORES, collective=True):
    import concourse.bass as bass  # noqa: F401
    import concourse.mybir as mybir
    from concourse import bacc, tile
    from concourse.tile import add_dep_helper

    f32 = mybir.dt.float32
    bf16 = mybir.dt.bfloat16
    Act = mybir.ActivationFunctionType
    Alu = mybir.AluOpType

    nc = bacc.Bacc("TRN2", num_devices=ncores, debug=False)
    x_t = nc.dram_tensor("x", [B_CORE, F], f32, kind="ExternalInput")
    g_t = nc.dram_tensor("gamma", [1, 1], f32, kind="ExternalInput")
    out_t = nc.dram_tensor("out", [1, 1], f32, kind="ExternalOutput")

    with tile.TileContext(nc) as tc:
        with tc.tile_pool(name="xp", bufs=7) as xp, \
             tc.tile_pool(name="scratch", bufs=2) as scp, \
             tc.tile_pool(name="small", bufs=3) as stp, \
             tc.tile_pool(name="tail", bufs=1) as tlp, \
             tc.tile_pool(name="ps", bufs=1, space="PSUM") as psp, \
             tc.tile_pool(name="dram", bufs=1, space="DRAM") as dram:

            # PSUM accumulators: part p lives at psum partition 32*p
            # (PE col tile_position constraint). Two 4-bank accumulators:
            # tile 0 -> S_a (AllReduce'd early, hidden under the DMA
            # stream; absorbs rank skew and warms the TOPSP), tiles
            # 1..7 -> S_b (AllReduce at the end, starts ~1us after its
            # doorbell since the collective stream is already warm).
            S_a = psp.tile([TILE_P, D], f32, tag="accA")
            S_b = psp.tile([TILE_P, D], f32, tag="accB")
            cc_in_a = dram.tile([NPARTS, D], f32)
            cc_out_a = dram.tile([NPARTS, D], f32)
            cc_in_b = dram.tile([NPARTS, D], f32)
            cc_out_b = dram.tile([NPARTS, D], f32)
            # AR1 covers only tile 0 so it triggers as early as possible:
            # its completion (which includes waiting for the most-skewed
            # rank) then lands well before AR2 is ready, so AR2 never
            # queues behind it on the collective stream.
            HALF = 1

            prev_sqrt = None
            prev_cast = None
            for i in range(NTILES):
                last = i == NTILES - 1
                # SWDGE DMA casts fp32 -> bf16 in-flight (free; PE wants
                # bf16 and the loss has ~1e3x precision headroom).
                # Last tile: split per part so its (fully exposed)
                # normalize chain starts at the first part boundary.
                xt = xp.tile([TILE_P, F], bf16, tag="xt")
                rows = x_t[i * TILE_P:(i + 1) * TILE_P, :]
                if last:
                    for p in range(NPARTS):
                        nc.gpsimd.dma_start(xt[:, p * D:(p + 1) * D],
                                            rows[:, p * D:(p + 1) * D])
                else:
                    nc.gpsimd.dma_start(xt[:], rows)

                # sum-of-squares per part, all on ACT (square + free
                # accumulator). Keeping the big elementwise ops OFF the
                # vector engine matters: DVE SBUF reads lock GpSimd out
                # of the port it uses for SWDGE descriptor rings, which
                # stalls the x-tile DMA stream.
                ss = stp.tile([TILE_P, NPARTS], f32, tag="ss")
                sqa = scp.tile([TILE_P, D], bf16, tag="sqa")
                norm = stp.tile([TILE_P, NPARTS], f32, tag="norm")
                r = stp.tile([TILE_P, NPARTS], f32, tag="r")
                r_bf = stp.tile([TILE_P, NPARTS], bf16, tag="r_bf")
                S_ps = S_a if i < HALF else S_b

                def mms_for_part(p, rbf_ap):
                    for j in range(NCHUNK):
                        nc.tensor.matmul(
                            S_ps[32 * p:32 * p + 1, j * MM_N:(j + 1) * MM_N],
                            lhsT=rbf_ap,
                            rhs=xt[:, p * D + j * MM_N:p * D + (j + 1) * MM_N],
                            start=(i == 0 or i == HALF),
                            stop=(i == HALF - 1 or i == NTILES - 1),
                            tile_position=(0, 32 * p))

                if not last:
                    for p in range(NPARTS):
                        a = nc.scalar.activation(
                            sqa[:], xt[:, p * D:(p + 1) * D], Act.Square,
                            accum_out=ss[:, p:p + 1])
                        if p == 0 and prev_sqrt is not None:
                            # pin ACT order: sqrt(i-1) must precede
                            # squares(i), else the scheduler makes r(i-1)
                            # wait on DMA(i)
                            add_dep_helper(
                                a.ins, prev_sqrt.ins, sync=False,
                                reason="sqrt(i-1) before squares(i)")
                    prev_sqrt = nc.scalar.sqrt(norm[:], ss[:])
                    nc.vector.reciprocal(r[:], norm[:])
                    prev_cast = nc.vector.tensor_copy(r_bf[:], r[:])
                    for p in range(NPARTS):
                        mms_for_part(p, r_bf[:, p:p + 1])
                else:
                    # per-part chain: square -> sqrt -> recip -> cast ->
                    # matmuls, so part p's work starts as soon as its
                    # quarter of the final DMA lands
                    pa = None
                    for p in range(NPARTS):
                        a = nc.scalar.activation(
                            sqa[:], xt[:, p * D:(p + 1) * D], Act.Square,
                            accum_out=ss[:, p:p + 1])
                        if p == 0 and prev_sqrt is not None:
                            add_dep_helper(a.ins, prev_sqrt.ins, sync=False,
                                           reason="sqrt(i-1) first")
                        if pa is not None:
                            add_dep_helper(a.ins, pa.ins, sync=False,
                                           reason="ACT part order")
                        pa = nc.scalar.sqrt(norm[:, p:p + 1], ss[:, p:p + 1])
                        nc.vector.reciprocal(r[:, p:p + 1], norm[:, p:p + 1])
                        nc.vector.tensor_copy(r_bf[:, p:p + 1], r[:, p:p + 1])
                        mms_for_part(p, r_bf[:, p:p + 1])

                if i == HALF - 1:
                    # first-half partial sums: ship out + AllReduce now,
                    # overlapped with the second half of the DMA stream
                    s_sba = tlp.tile([TILE_P, D], f32, tag="s_sba")
                    nc.vector.tensor_copy(s_sba[:], S_a[:])
                    for p in range(NPARTS):
                        nc.sync.dma_start(cc_in_a[p:p + 1, :],
                                          s_sba[32 * p:32 * p + 1, :])
                    if collective:
                        nc.gpsimd.collective_compute(
                            "AllReduce", Alu.add,
                            replica_groups=[list(range(ncores))],
                            ins=[cc_in_a.opt()], outs=[cc_out_a.opt()])
                    else:
                        nc.sync.dma_start(cc_out_a[:], cc_in_a[:])

            # ---- second-half partial sums: AllReduce over 8 cores ----
            # one full-width PSUM->SBUF copy (rows besides 0/32/64/96 are
            # junk but harmless) instead of 4 serial row copies
            s_sb = tlp.tile([TILE_P, D], f32, tag="s_sb")
            nc.scalar.copy(s_sb[:, :D // 2], S_b[:, :D // 2])
            nc.vector.tensor_copy(s_sb[:, D // 2:], S_b[:, D // 2:])

            for p in range(NPARTS):
                eng = nc.sync if p % 2 == 0 else nc.scalar
                eng.dma_start(cc_in_b[p:p + 1, :],
                              s_sb[32 * p:32 * p + 1, :])
            ar2 = None
            if collective:
                ar2 = nc.gpsimd.collective_compute(
                    "AllReduce", Alu.add,
                    replica_groups=[list(range(ncores))],
                    ins=[cc_in_b.opt()], outs=[cc_out_b.opt()])
            else:
                nc.sync.dma_start(cc_out_b[:], cc_in_b[:])

            # reload both summed halves as bf16 (cast in DMA) and add
            sfa = tlp.tile([NPARTS, D], bf16, tag="sfa")
            ld_a = nc.gpsimd.dma_start(sfa[:], cc_out_a[:])
            if ar2 is not None:
                # keep gpsimd free to fire the AR2 doorbell before it
                # blocks on AR1's output
                add_dep_helper(ld_a.ins, ar2.ins, sync=False,
                               reason="AR2 doorbell before sfa load")
            sfb = tlp.tile([NPARTS, D], bf16, tag="sfb")
            nc.gpsimd.dma_start(sfb[:], cc_out_b[:])

            # ---- replicated tail: loss scalar ----
            ones4 = tlp.tile([NPARTS, 1], bf16, tag="ones4")
            nc.vector.memset(ones4[:], 1.0)
            ones4f = tlp.tile([NPARTS, 1], f32, tag="ones4f")
            nc.vector.memset(ones4f[:], 1.0)

            # t = sum_p S_p: ones-matmuls PSUM-accumulate sfa (available
            # at AR1-end, i.e. before AR2 finishes) then sfb — the
            # sfa+sfb add is off the t critical path entirely
            t_ps = psp.tile([1, D], f32, tag="accA")
            for half, sf in ((0, sfa), (1, sfb)):
                for j in range(NCHUNK):
                    nc.tensor.matmul(
                        t_ps[0:1, j * MM_N:(j + 1) * MM_N],
                        lhsT=ones4[:],
                        rhs=sf[:, j * MM_N:(j + 1) * MM_N],
                        start=(half == 0), stop=(half == 1))

            # B2 = sum((sfa+sfb)^2): DVE add, then ACT square+accum
            sfull = tlp.tile([NPARTS, D], bf16, tag="sfull")
            nc.vector.tensor_add(sfull[:], sfa[:], sfb[:])
            sq_tail = tlp.tile([NPARTS, D], bf16, tag="sq_tail")
            ssum = tlp.tile([NPARTS, 1], f32, tag="ssum")
            nc.scalar.activation(sq_tail[:], sfull[:], Act.Square,
                                 accum_out=ssum[:])
            b2_ps = psp.tile([1, 1], f32, tag="accB")
            nc.tensor.matmul(b2_ps[:], lhsT=ones4f[:], rhs=ssum[:],
                             start=True, stop=True)

            t_sq = tlp.tile([1, D], f32, tag="t_sq")
            a_sb = tlp.tile([1, 1], f32, tag="a_sb")
            nc.scalar.activation(t_sq[:], t_ps[:], Act.Square,
                                 accum_out=a_sb[:])

            # loss = ((A - 3*B2) / B^2 + 2P) / 2 * gamma
            g_sb = tlp.tile([1, 1], f32, tag="g_sb")
            nc.sync.dma_start(g_sb[:], g_t[:])
            tmp = tlp.tile([1, 1], f32, tag="tmp")
            nc.vector.tensor_scalar(
                out=tmp[:], in0=b2_ps[:], scalar1=-3.0, scalar2=None,
                op0=Alu.mult)
            tt = tlp.tile([1, 1], f32, tag="tt")
            nc.vector.tensor_add(tt[:], tmp[:], a_sb[:])
            l0 = tlp.tile([1, 1], f32, tag="l0")
            nc.vector.tensor_scalar(
                out=l0[:], in0=tt[:],
                scalar1=1.0 / (2.0 * float(B) * float(B)),
                scalar2=float(NPARTS),
                op0=Alu.mult, op1=Alu.add)
            loss = tlp.tile([1, 1], f32, tag="loss")
            nc.vector.tensor_mul(loss[:], l0[:], g_sb[:])
            nc.sync.dma_start(out_t[:], loss[:])

    nc.compile()
    return nc


def _get_nc():
    if "nc" not in _cache:
        _cache["nc"] = _build()
    return _cache["nc"]


def kernel(x, gamma, **run_kwargs):
    from concourse import bass_utils

    x = np.ascontiguousarray(np.asarray(x, dtype=np.float32))
    gamma = np.asarray(gamma, dtype=np.float32).reshape(1, 1)
    assert x.shape == (B, F), x.shape

    nc = _get_nc()
    in_maps = [
        {"x": x[c * B_CORE:(c + 1) * B_CORE], "gamma": gamma}
        for c in range(NCORES)
    ]
    res = bass_utils.run_bass_kernel_spmd(
        nc, in_maps, core_ids=list(range(NCORES)), **run_kwargs)
    out = np.asarray(res.results[0]["out"], dtype=np.float32).reshape(1)
    if run_kwargs.get("trace"):
        _cache["last_results"] = res
    return out



# revision 11
# speedup vs baseline: 1.0754x; 1.0754x over previous
"""Trainium2 Bass kernel for nn_C3S_RegularLoss.

reference:
    xr = x.reshape(B, P, D); xn = xr / ||xr||_2(axis=-1)
    s = mean_b(xn)                     # (P, D)
    corr = s @ s.T                     # (P, P)
    loss = (sum(corr) - 3*trace(corr) + 2P) / 2 * gamma

Reformulated without the corr matrix, with S = sum_b xn (sum, not mean):
    sum(corr)   = ||sum_p S_p||^2 / B^2 = A' / B^2
    trace(corr) = sum_p ||S_p||^2 / B^2 = B2' / B^2
    loss = (A' - 3*B2') * gamma/(2 B^2) + P*gamma

Sharding: data-parallel over the batch dim, 8 cores x 1024 rows.
Each core accumulates S_partial = sum_b r_b * x_b for all 8 row-tiles
into ONE PSUM accumulator (r = 1/||x_part|| stationary operand), ships
the 4 used PSUM rows as bf16, AllReduces (4,2048) bf16, then computes
the scalar tail on a (32,256) reshaped view so all reductions use many
partitions and the A'-3*B2' subtraction happens inside PSUM.

A tiny dummy AllReduce fired at t~10us absorbs the one-time ~16us
collective-stream setup so the real AllReduce's trigger->mesh latency
is ~1us.
"""

import os
import sys

sys.path.insert(0, "/opt/trn_rl_repo")
os.environ.setdefault("MYCRO_LOCAL_CACHE", "1")

import numpy as np

B, F = 8192, 8192
NPARTS = 4
D = F // NPARTS                 # 2048
NCORES = 8
B_CORE = B // NCORES            # 1024
TILE_P = 128
NTILES = B_CORE // TILE_P       # 8
MM_N = 512                      # moving free dim per matmul
NCHUNK = D // MM_N              # 4
TJ = 8                          # tail reshape: (32, 256) = ((j p), c)
TC = D // TJ                    # 256

_cache = {}


def _build(ncores=NCORES, collective=True):
    import concourse.bass as bass  # noqa: F401
    import concourse.mybir as mybir
    from concourse import bacc, tile
    from concourse.tile import add_dep_helper

    f32 = mybir.dt.float32
    bf16 = mybir.dt.bfloat16
    Act = mybir.ActivationFunctionType
    Alu = mybir.AluOpType
    Ax = mybir.AxisListType

    nc = bacc.Bacc("TRN2", num_devices=ncores, debug=False)
    x_t = nc.dram_tensor("x", [B_CORE, F], f32, kind="ExternalInput")
    g_t = nc.dram_tensor("gamma", [1, 1], f32, kind="ExternalInput")
    out_t = nc.dram_tensor("out", [1, 1], f32, kind="ExternalOutput")

    with tile.TileContext(nc) as tc:
        with tc.tile_pool(name="xp", bufs=7) as xp, \
             tc.tile_pool(name="scratch", bufs=2) as scp, \
             tc.tile_pool(name="small", bufs=3) as stp, \
             tc.tile_pool(name="tail", bufs=1) as tlp, \
             tc.tile_pool(name="ps", bufs=1, space="PSUM") as psp, \
             tc.tile_pool(name="dram", bufs=1, space="DRAM") as dram:

            # single PSUM accumulator: part p lives at psum partition
            # 32*p (PE col tile_position constraint); all 8 row-tiles
            # accumulate into it (start at tile 0, stop at tile 7).
            S_ps = psp.tile([TILE_P, D], f32, tag="acc")
            X_ps = psp.tile([1, 1], f32, tag="X")

            cc_in = dram.tile([NPARTS, D], bf16)
            cc_out = dram.tile([NPARTS, D], bf16)
            cc_w_in = dram.tile([1, 16], f32)
            cc_w_out = dram.tile([1, 16], f32)

            # ---- head: constants + warmup-collective feed ----
            g_sb = tlp.tile([1, 1], f32, tag="g_sb")
            nc.sync.dma_start(g_sb[:], g_t[:])
            w_in = tlp.tile([1, 16], f32, tag="w_in")
            nc.vector.memset(w_in[:], 0.0)
            nc.sync.dma_start(cc_w_in[:], w_in[:])

            # loss = X * gamma/(2 B^2) + P*gamma, X = A' - 3*B2'
            gscale = tlp.tile([1, 1], f32, tag="gscale")
            nc.vector.tensor_scalar(
                out=gscale[:], in0=g_sb[:],
                scalar1=1.0 / (2.0 * float(B) * float(B)), scalar2=None,
                op0=Alu.mult)
            gp = tlp.tile([1, 1], f32, tag="gp")
            nc.vector.tensor_scalar(
                out=gp[:], in0=g_sb[:], scalar1=float(NPARTS), scalar2=None,
                op0=Alu.mult)

            ones32 = tlp.tile([32, 1], f32, tag="ones32")
            nc.vector.memset(ones32[:], 1.0)
            neg3 = tlp.tile([32, 1], f32, tag="neg3")
            nc.vector.memset(neg3[:], -3.0)

            # ---- main loop over 8 row-tiles ----
            prev_ars = None
            warm_done = False
            for i in range(NTILES):
                last = i == NTILES - 1
                # SWDGE DMA casts fp32 -> bf16 in-flight. First tile:
                # per-part split so the first doorbell rings ~2us
                # earlier (smaller descriptor batch). Last tile: per
                # part so each part's normalize chain starts at its
                # part boundary.
                xt = xp.tile([TILE_P, F], bf16, tag="xt")
                rows = x_t[i * TILE_P:(i + 1) * TILE_P, :]
                if i == 0 or last:
                    for p in range(NPARTS):
                        nc.gpsimd.dma_start(xt[:, p * D:(p + 1) * D],
                                            rows[:, p * D:(p + 1) * D])
                else:
                    nc.gpsimd.dma_start(xt[:], rows)

                if not warm_done:
                    # dummy warmup AllReduce: absorbs the one-time
                    # collective-stream setup (~16us) + mesh crawl
                    # under the DMA stream, so the real AllReduce's
                    # trigger->mesh-begin is ~1us.
                    if collective:
                        nc.gpsimd.collective_compute(
                            "AllReduce", Alu.add,
                            replica_groups=[list(range(ncores))],
                            ins=[cc_w_in.opt()], outs=[cc_w_out.opt()])
                    warm_done = True

                # sum-of-squares per part on ACT (square + free
                # accumulator); r = 1/sqrt(ss) fused on ACT with bf16
                # output. Big elementwise work stays OFF the vector
                # engine mid-stream (DVE SBUF reads lock GpSimd out of
                # the SWDGE descriptor-ring ports).
                ss = stp.tile([TILE_P, NPARTS], f32, tag="ss")
                sqa = scp.tile([TILE_P, D], bf16, tag="sqa")
                r_bf = stp.tile([TILE_P, NPARTS], bf16, tag="r_bf")

                def mms_for_part(p, rbf_ap):
                    for j in range(NCHUNK):
                        nc.tensor.matmul(
                            S_ps[32 * p:32 * p + 1, j * MM_N:(j + 1) * MM_N],
                            lhsT=rbf_ap,
                            rhs=xt[:, p * D + j * MM_N:p * D + (j + 1) * MM_N],
                            start=(i == 0),
                            stop=(i == NTILES - 1),
                            tile_position=(0, 32 * p))

                if not last:
                    for p in range(NPARTS):
                        a = nc.scalar.activation(
                            sqa[:], xt[:, p * D:(p + 1) * D], Act.Square,
                            accum_out=ss[:, p:p + 1])
                        if p == 0 and prev_ars is not None:
                            # pin ACT order: ars(i-1) must precede
                            # squares(i), else the scheduler makes
                            # r(i-1) wait on DMA(i)
                            add_dep_helper(
                                a.ins, prev_ars.ins, sync=False,
                                reason="ars(i-1) before squares(i)")
                    prev_ars = nc.scalar.activation(
                        r_bf[:], ss[:], Act.Abs_reciprocal_sqrt)
                    for p in range(NPARTS):
                        mms_for_part(p, r_bf[:, p:p + 1])
                else:
                    # last tile: parts 0,2 square on ACT; parts 1,3 on
                    # DVE (mult+reduce) so the two engines pipeline
                    # behind the final DMAs. Per-part ars -> matmuls.
                    sqv = scp.tile([TILE_P, D], bf16, tag="sqv")
                    acts = {}
                    for p in (0, 2):
                        a = nc.scalar.activation(
                            sqa[:], xt[:, p * D:(p + 1) * D], Act.Square,
                            accum_out=ss[:, p:p + 1])
                        if p == 0 and prev_ars is not None:
                            add_dep_helper(a.ins, prev_ars.ins, sync=False,
                                           reason="ars(i-1) first")
                        acts[f"sq{p}"] = a
                    for p in (1, 3):
                        nc.vector.tensor_mul(
                            sqv[:], xt[:, p * D:(p + 1) * D],
                            xt[:, p * D:(p + 1) * D])
                        nc.vector.tensor_reduce(
                            out=ss[:, p:p + 1], in_=sqv[:], axis=Ax.X,
                            op=Alu.add)
                    for p in range(NPARTS):
                        acts[f"ars{p}"] = nc.scalar.activation(
                            r_bf[:, p:p + 1], ss[:, p:p + 1],
                            Act.Abs_reciprocal_sqrt)
                        mms_for_part(p, r_bf[:, p:p + 1])
                    # ACT order: sq0 -> ars0 -> ars1 -> sq2 -> ars2 -> ars3
                    chain = ["sq0", "ars0", "ars1", "sq2", "ars2", "ars3"]
                    for a, b in zip(chain, chain[1:]):
                        add_dep_helper(acts[b].ins, acts[a].ins, sync=False,
                                       reason=f"ACT order {a}->{b}")

            # ---- ship the 4 used PSUM rows out as bf16 ----
            # Engines need unit partition step, so copy full-width
            # (junk rows besides 0/32/64/96 are harmless) in column
            # quarters alternated across ACT and DVE; the partition
            # stride lives in the DMA access pattern instead.
            s4 = tlp.tile([TILE_P, D], bf16, tag="s4")
            Q = D // 4
            for q in range(4):
                eng = nc.scalar if q % 2 == 0 else nc.vector
                if q % 2 == 0:
                    eng.copy(s4[:, q * Q:(q + 1) * Q],
                             S_ps[:, q * Q:(q + 1) * Q])
                else:
                    eng.tensor_copy(s4[:, q * Q:(q + 1) * Q],
                                    S_ps[:, q * Q:(q + 1) * Q])
            nc.sync.dma_start(cc_in[:], s4[0:3 * 32 + 1:32, :])

            ar = None
            if collective:
                ar = nc.gpsimd.collective_compute(
                    "AllReduce", Alu.add,
                    replica_groups=[list(range(ncores))],
                    ins=[cc_in.opt()], outs=[cc_out.opt()])
            else:
                nc.sync.dma_start(cc_out[:], cc_in[:])

            # ---- replicated tail on a (32,256) view ----
            # Tc[k, p*64+c] = S[p, k*64+c]: part p is a 64-wide column
            # block, so the cross-part sum is column-wise DVE adds (one
            # partition base) and every reduction uses 32 partitions.
            Tc = tlp.tile([32, 4 * 64], bf16, tag="Tc")
            for p in range(NPARTS):
                eng = nc.sync if p % 2 == 0 else nc.scalar
                eng.dma_start(
                    Tc[:, p * 64:(p + 1) * 64],
                    cc_out[p:p + 1, :].rearrange("o (k c) -> (o k) c", k=32))

            # B2' = sum(Tc^2)
            sqB = tlp.tile([32, 4 * 64], bf16, tag="sqB")
            ssB = tlp.tile([32, 1], f32, tag="ssB")
            nc.scalar.activation(sqB[:], Tc[:], Act.Square,
                                 accum_out=ssB[:])
            # t32 = sum_p part blocks  -> A' = sum(t32^2)
            u32 = tlp.tile([32, 64], bf16, tag="u32")
            v32 = tlp.tile([32, 64], bf16, tag="v32")
            t32 = tlp.tile([32, 64], bf16, tag="t32")
            nc.vector.tensor_add(u32[:], Tc[:, 0:64], Tc[:, 64:128])
            nc.vector.tensor_add(v32[:], Tc[:, 128:192], Tc[:, 192:256])
            nc.vector.tensor_add(t32[:], u32[:], v32[:])
            sqA = tlp.tile([32, 64], bf16, tag="sqA")
            ssA = tlp.tile([32, 1], f32, tag="ssA")
            nc.scalar.activation(sqA[:], t32[:], Act.Square,
                                 accum_out=ssA[:])
            # X = A' - 3*B2' accumulated inside one PSUM cell
            nc.tensor.matmul(X_ps[:], lhsT=ones32[:], rhs=ssA[:],
                             start=True, stop=False)
            nc.tensor.matmul(X_ps[:], lhsT=neg3[:], rhs=ssB[:],
                             start=False, stop=True)
            # loss = gscale * X + gp, single ACT op
            loss = tlp.tile([1, 1], f32, tag="loss")
            nc.scalar.activation(loss[:], X_ps[:], Act.Identity,
                                 bias=gp[0:1, 0:1], scale=gscale[0:1, 0:1])
            nc.sync.dma_start(out_t[:], loss[:])

    nc.compile()
    return nc


def _get_nc():
    if "nc" not in _cache:
        _cache["nc"] = _build()
    return _cache["nc"]


def kernel(x, gamma, **run_kwargs):
    from concourse import bass_utils

    x = np.ascontiguousarray(np.asarray(x, dtype=np.float32))
    gamma = np.asarray(gamma, dtype=np.float32).reshape(1, 1)
    assert x.shape == (B, F), x.shape

    nc = _get_nc()
    in_maps = [
        {"x": x[c * B_CORE:(c + 1) * B_CORE], "gamma": gamma}
        for c in range(NCORES)
    ]
    res = bass_utils.run_bass_kernel_spmd(
        nc, in_maps, core_ids=list(range(NCORES)), **run_kwargs)
    out = np.asarray(res.results[0]["out"], dtype=np.float32).reshape(1)
    if run_kwargs.get("trace"):
        _cache["last_results"] = res
    return out
